# revision 1
# baseline (speedup 1.0000x reference)
"""Trainium2 Bass kernel for nn_BucketedGoWatti (sparse windowed attention pooling).

Math (B=4, L=4096, T=32, DH=1024, DG=256, DP=256, WIN=1024, STRIDE=256, W=13):
  q  = G @ Wq_core;  k = H @ Wk_core (window-independent)
  logits[b,w,t,l] = slice of global  s * (q @ Wk_core^T) @ H^T
  alpha = softmax in window; Zw[b,t,w,:] = alpha @ Hw
  wlog[b,t,w] = Zw . qw2,  qw2 = (G@Wq_win) @ Wk_win^T * DH^-0.5
  Z = softmax_w(wlog) @ Zw   (tiny; done on host at gather time)

Sharding: core c -> batch b=c//2, window half c%2 (even: windows 0-6 over
l in [0,2560); odd: windows 6-12 over l in [1536,4096); window 6 duplicated
so all 8 cores run one SPMD program shape). Cross-window combine on host.

Precision: big matmuls in float32r (~1.5e-4 rel); qw2 path bf16 (negligible
through the 13-way combine softmax); softmax/normalization fp32.
"""
import numpy as np
import ml_dtypes
from contextlib import ExitStack

import concourse.bacc as bacc
import concourse.tile as tile
import concourse.mybir as mybir
import concourse.masks as masks
from concourse.bass_utils import run_bass_kernel_spmd

F32 = mybir.dt.float32
F32R = mybir.dt.float32r
BF16 = mybir.dt.bfloat16
ActFn = mybir.ActivationFunctionType
Alu = mybir.AluOpType

B, L, T = 4, 4096, 32
DH, DG, DP = 1024, 256, 256
WIN, STRIDE = 1024, 256
W = (L - WIN) // STRIDE + 1          # 13
SPAN = 2560                          # per-core l-span
NLT = SPAN // 128                    # 20 l-tiles
NCH = SPAN // 256                    # 10 logits chunks of 256
WLOC = 7                             # windows per core
NDT = DH // 128                      # 8 d-tiles
S_CORE = 1.0 / float(np.sqrt(DP))
S_WIN = 1.0 / float(np.sqrt(DH))

_CACHE = {}


def _build(with_mask: bool, stage: int = 99):
    nc = bacc.Bacc("TRN2", debug=False, target_bir_lowering=False)

    Hn_d = nc.dram_tensor("Hn", [SPAN, DH], F32R, kind="ExternalInput")
    HT_d = nc.dram_tensor("HT", [DH, SPAN], F32R, kind="ExternalInput")
    GT_d = nc.dram_tensor("GT", [DG, T], F32R, kind="ExternalInput")
    Wqc_d = nc.dram_tensor("Wqc", [DG, DP], F32R, kind="ExternalInput")
    WkcT_d = nc.dram_tensor("WkcT", [DP, DH], F32R, kind="ExternalInput")
    Wqw_d = nc.dram_tensor("Wqw", [DG, DH], F32R, kind="ExternalInput")
    WkwT_d = nc.dram_tensor("WkwT", [DH, DH], BF16, kind="ExternalInput")
    if with_mask:
        mb_d = nc.dram_tensor("maskbias", [1, SPAN], F32R, kind="ExternalInput")
        ones_d = nc.dram_tensor("onesrow", [1, T], F32R, kind="ExternalInput")
    zw_d = nc.dram_tensor("Zw_out", [WLOC * T, DH], F32, kind="ExternalOutput")
    wl_d = nc.dram_tensor("wlog_out", [T, WLOC], F32, kind="ExternalOutput")

    with tile.TileContext(nc) as tc, ExitStack() as ctx:
        const = ctx.enter_context(tc.tile_pool(name="const", bufs=1))
        hpool = ctx.enter_context(tc.tile_pool(name="hpool", bufs=16))
        htp = ctx.enter_context(tc.tile_pool(name="htp", bufs=12))
        sb = ctx.enter_context(tc.tile_pool(name="sb", bufs=1))
        sexp = ctx.enter_context(tc.tile_pool(name="sexp", bufs=1))
        pj = ctx.enter_context(tc.tile_pool(name="pj", bufs=2, space="PSUM"))
        lg = ctx.enter_context(tc.tile_pool(name="lg", bufs=2, space="PSUM"))
        zp = ctx.enter_context(tc.tile_pool(name="zp", bufs=4, space="PSUM"))

        # ---- small resident inputs ----
        ident = const.tile([128, 128], F32, tag="ident")
        masks.make_identity(nc, ident[:])
        gt = const.tile([128, 2 * T], F32R, tag="gt")
        wqc = const.tile([128, 2 * DP], F32R, tag="wqc")
        wkcT = const.tile([128, 2 * DH], F32R, tag="wkcT")
        wqw = const.tile([128, 2 * DH], F32R, tag="wqw")
        wkwT = const.tile([128, NDT * DH], BF16, tag="wkwT")
        for g in range(2):
            nc.gpsimd.dma_start(gt[:, g * T:(g + 1) * T], GT_d.ap()[g * 128:(g + 1) * 128, :])
            nc.gpsimd.dma_start(wqc[:, g * DP:(g + 1) * DP], Wqc_d.ap()[g * 128:(g + 1) * 128, :])
            nc.gpsimd.dma_start(wkcT[:, g * DH:(g + 1) * DH], WkcT_d.ap()[g * 128:(g + 1) * 128, :])
            nc.gpsimd.dma_start(wqw[:, g * DH:(g + 1) * DH], Wqw_d.ap()[g * 128:(g + 1) * 128, :])

        if with_mask:
            mbias = const.tile([1, SPAN], F32R, tag="mbias")
            onesr = const.tile([1, T], F32R, tag="onesr")
            nc.gpsimd.dma_start(mbias[:], mb_d.ap())
            nc.gpsimd.dma_start(onesr[:], ones_d.ap())

        # ---- q^T then qk^T ----
        qT = []
        for p in range(2):
            ps_ = pj.tile([128, 512], F32, tag="pj")
            for g in range(2):
                nc.tensor.matmul(ps_[:, :T], wqc[:, g * DP + p * 128:g * DP + (p + 1) * 128],
                                 gt[:, g * T:(g + 1) * T], start=(g == 0), stop=(g == 1))
            t_ = sb.tile([128, T], F32R, tag=f"qT{p}")
            nc.scalar.activation(t_[:], ps_[:, :T], ActFn.Identity, scale=S_CORE)
            qT.append(t_)
        qkT = []
        for i in range(NDT):
            ps_ = pj.tile([128, 512], F32, tag="pj")
            for p in range(2):
                nc.tensor.matmul(ps_[:, :T], wkcT[:, p * DH + i * 128:p * DH + (i + 1) * 128],
                                 qT[p][:], start=(p == 0), stop=(p == 1))
            t_ = sb.tile([128, T], F32R, tag=f"qkT{i}")
            nc.vector.tensor_copy(t_[:], ps_[:, :T])
            qkT.append(t_)
        if stage == 1:
            dbg = sb.tile([128, NDT * T], F32, tag="dbg")
            for i in range(NDT):
                nc.vector.tensor_copy(dbg[:, i * T:(i + 1) * T], qkT[i][:].bitcast(F32))
            nc.sync.dma_start(zw_d.ap()[:128, :NDT * T], dbg[:])

        # ---- logits chunks + exp (+ per-chunk sums) ----
        hn = []
        if stage >= 2:
            expLs, csums = [], []
            for c in range(NCH):
                ec_ = sexp.tile([T, 256], F32, tag=f"expL{c}")
                cs_ = sexp.tile([T, 1], F32, tag=f"csum{c}")
                expLs.append(ec_)
                csums.append(cs_)
            ht = {}
            for cc in range(SPAN // 512):
                for i in range(NDT):
                    t_ = htp.tile([128, 512], F32R, tag="ht")
                    nc.sync.dma_start(t_[:], HT_d.ap()[i * 128:(i + 1) * 128,
                                                       cc * 512:(cc + 1) * 512])
                    ht[(cc, i)] = t_
                if cc == 0 and stage >= 5:
                    for j in range(NLT):
                        t_ = hpool.tile([128, DH], F32R, tag="hn")
                        nc.scalar.dma_start(t_[:], Hn_d.ap()[j * 128:(j + 1) * 128, :])
                        hn.append(t_)
                    for e in range(NDT):
                        nc.gpsimd.dma_start(wkwT[:, e * DH:(e + 1) * DH],
                                            WkwT_d.ap()[e * 128:(e + 1) * 128, :])
            for cc in range(SPAN // 512):
                ps_ = lg.tile([T, 512], F32, tag="lg")
                for i in range(NDT):
                    nc.tensor.matmul(ps_[:], qkT[i][:], ht[(cc, i)][:],
                                     start=(i == 0), stop=(i == NDT - 1 and not with_mask))
                if with_mask:
                    nc.tensor.matmul(ps_[:], onesr[:], mbias[:, cc * 512:(cc + 1) * 512],
                                     start=False, stop=True)
                for u in range(2):
                    c = 2 * cc + u
                    nc.scalar.activation(expLs[c][:], ps_[:, u * 256:(u + 1) * 256],
                                         ActFn.Exp, accum_out=csums[c][:])
            if stage == 2:
                nc.sync.dma_start(zw_d.ap()[:T, :256], expLs[0][:])

        # ---- transpose expL into [l, t] f32r tiles; denominators ----
        if stage >= 3:
            expLT = []
            for j in range(NLT):
                ps_ = pj.tile([128, 512], F32, tag="pj")
                nc.tensor.transpose(ps_[:, :T], expLs[j // 2][:, (j % 2) * 128:(j % 2) * 128 + 128],
                                    ident[:T, :T])
                t_ = sb.tile([128, T], F32R, tag=f"eT{j}")
                nc.vector.tensor_copy(t_[:], ps_[:, :T])
                expLT.append(t_)
            recs = []
            for j in range(WLOC):
                d0_ = sexp.tile([T, 1], F32, tag=f"d0_{j}")
                d1_ = sexp.tile([T, 1], F32, tag=f"d1_{j}")
                rc_ = sexp.tile([T, 1], F32, tag=f"rc_{j}")
                nc.vector.tensor_add(d0_[:], csums[j][:], csums[j + 1][:])
                nc.vector.tensor_add(d1_[:], csums[j + 2][:], csums[j + 3][:])
                nc.vector.tensor_add(d0_[:], d0_[:], d1_[:])
                nc.vector.reciprocal(rc_[:], d0_[:])
                recs.append(rc_)
            if stage == 3:
                dbg = sb.tile([128, 2 * T], F32, tag="dbg")
                nc.vector.tensor_copy(dbg[:, :T], expLT[0][:].bitcast(F32))
                nc.vector.tensor_copy(dbg[:, T:2 * T], expLT[1][:].bitcast(F32))
                nc.sync.dma_start(zw_d.ap()[:128, :2 * T], dbg[:])
                nc.sync.dma_start(wl_d.ap()[:, :1], recs[0][:])

        # ---- qw -> qw^T(bf16) -> qw2 ----
        if stage >= 4:
            qw = sb.tile([T, DH], F32, tag="qw")
            for h in range(2):
                ps_ = zp.tile([T, 512], F32, tag="zp")
                for g in range(2):
                    nc.tensor.matmul(ps_[:], gt[:, g * T:(g + 1) * T],
                                     wqw[:, g * DH + h * 512:g * DH + (h + 1) * 512],
                                     start=(g == 0), stop=(g == 1))
                nc.scalar.activation(qw[:, h * 512:(h + 1) * 512], ps_[:], ActFn.Identity,
                                     scale=S_WIN)
            qwT = []
            for e in range(NDT):
                ps_ = pj.tile([128, 512], F32, tag="pj")
                nc.tensor.transpose(ps_[:, :T], qw[:, e * 128:(e + 1) * 128], ident[:T, :T])
                t_ = sb.tile([128, T], BF16, tag=f"qwT{e}")
                nc.vector.tensor_copy(t_[:], ps_[:, :T])
                qwT.append(t_)
            qw2 = sb.tile([T, DH], F32, tag="qw2")
            for h in range(2):
                ps_ = zp.tile([T, 512], F32, tag="zp")
                for e in range(NDT):
                    nc.tensor.matmul(ps_[:], qwT[e][:],
                                     wkwT[:, e * DH + h * 512:e * DH + (h + 1) * 512],
                                     start=(e == 0), stop=(e == NDT - 1))
                nc.scalar.activation(qw2[:, h * 512:(h + 1) * 512], ps_[:], ActFn.Identity)
            if stage == 4:
                nc.sync.dma_start(zw_d.ap()[:T, :DH], qw2[:])

        # ---- Zw per window (normalized in PSUM->SBUF copy), wlog inline ----
        if stage >= 5:
            wlog = sexp.tile([T, WLOC], F32, tag="wlog")
            scratch = sexp.tile([T, DH], F32, tag="scratch")
            for j in range(WLOC):
                t_ = sb.tile([T, DH], F32, tag="zw")
                ps_a = zp.tile([T, 512], F32, tag="zp")
                ps_b = zp.tile([T, 512], F32, tag="zp")
                pss = [ps_a, ps_b]
                for k in range(8):
                    for h in range(2):
                        nc.tensor.matmul(pss[h][:], expLT[2 * j + k][:],
                                         hn[2 * j + k][:, h * 512:(h + 1) * 512],
                                         start=(k == 0), stop=(k == 7))
                for h in range(2):
                    nc.vector.tensor_scalar_mul(t_[:, h * 512:(h + 1) * 512], pss[h][:],
                                                recs[j][:])
                nc.sync.dma_start(zw_d.ap()[j * T:(j + 1) * T, :], t_[:])
                if stage >= 7:
                    nc.vector.tensor_mul(scratch[:], t_[:], qw2[:])
                    nc.vector.reduce_sum(wlog[:, j:j + 1], scratch[:],
                                         axis=mybir.AxisListType.X)
            if stage >= 7:
                nc.gpsimd.dma_start(wl_d.ap(), wlog[:])

    nc.compile()
    return nc


def kernel(H, G, Wq_core, Wk_core, Wq_win, Wk_win, attn_mask):
    H = np.asarray(H, dtype=np.float32)
    G = np.asarray(G, dtype=np.float32)
    Wq_core = np.asarray(Wq_core, dtype=np.float32)
    Wk_core = np.asarray(Wk_core, dtype=np.float32)
    Wq_win = np.asarray(Wq_win, dtype=np.float32)
    Wk_win = np.asarray(Wk_win, dtype=np.float32)
    mask = np.asarray(attn_mask).astype(bool)

    with_mask = not bool(mask.all())
    key = ("k", with_mask)
    if key not in _CACHE:
        _CACHE[key] = _build(with_mask)
    nc = _CACHE[key]

    WkcT = np.ascontiguousarray(Wk_core.T)
    WkwT = np.ascontiguousarray(Wk_win.T).astype(ml_dtypes.bfloat16)

    in_maps = []
    for c in range(8):
        b, half = c // 2, c % 2
        lo = 0 if half == 0 else L - SPAN
        im = {
            "Hn": np.ascontiguousarray(H[b, lo:lo + SPAN, :]),
            "HT": np.ascontiguousarray(H[b].T[:, lo:lo + SPAN]),
            "GT": np.ascontiguousarray(G[b].T),
            "Wqc": Wq_core,
            "WkcT": WkcT,
            "Wqw": Wq_win,
            "WkwT": WkwT,
        }
        if with_mask:
            im["maskbias"] = np.where(mask[b, lo:lo + SPAN], 0.0, -1e9).astype(np.float32)[None, :]
            im["onesrow"] = np.ones((1, T), dtype=np.float32)
        in_maps.append(im)

    import os
    prof_dir = os.environ.get("BGW_PROFILE_DIR")
    if prof_dir:
        res = run_bass_kernel_spmd(nc, in_maps, core_ids=list(range(8)),
                                   trace=True, tmpdir=prof_dir)
    else:
        res = run_bass_kernel_spmd(nc, in_maps, core_ids=list(range(8)))
    kernel._last_result = res

    # ---- host combine: tiny cross-window softmax over W=13 ----
    Z = np.empty((B, T, DH), dtype=np.float32)
    for b in range(B):
        zw_full = np.empty((W, T, DH), dtype=np.float32)
        wl_full = np.empty((T, W), dtype=np.float32)
        for half in range(2):
            r = res.results[2 * b + half]
            zw = r["Zw_out"].reshape(WLOC, T, DH)
            wl = r["wlog_out"]
            w0 = 0 if half == 0 else W - WLOC
            zw_full[w0:w0 + WLOC] = zw
            wl_full[:, w0:w0 + WLOC] = wl
        m = wl_full.max(axis=1, keepdims=True)
        e = np.exp(wl_full - m)
        wsm = e / e.sum(axis=1, keepdims=True)          # [T, W]
        Z[b] = np.einsum("tw,wtd->td", wsm, zw_full)
    return Z



# revision 4
# speedup vs baseline: 3.5689x; 3.5689x over previous
"""Trainium2 Bass kernel for nn_BucketedGoWatti (sparse windowed attention pooling).

Math (B=4, L=4096, T=32, DH=1024, DG=256, DP=256, WIN=1024, STRIDE=256, W=13):
  q  = G @ Wq_core;  logits[b,t,l] = (q @ Wk_core^T) . H[b,l]  (window-independent)
  alpha = softmax of logits restricted to window; Zw[b,t,w,:] = alpha @ Hw
  Since windows are 4 consecutive 256-chunks, Zw[w] = (P[w]+P[w+1]+P[w+2]+P[w+3])/den
  with P[c] = sum_{l in chunk c} exp(logit[t,l]) * H[l,:]  and den from per-chunk
  exp-sums. Device computes P[c] + csum[c] only; window composition, the tiny
  cross-window softmax (qw2 = (G@Wq_win)@Wk_win^T) and the final combine run on host.

Sharding: core c -> batch b=c//2, l-half c%2 (disjoint 2048 rows of H, zero halo).
Each core streams H once in each orientation (bf16): HT (d-major) for logits,
Hn (l-major) for P. Host pre-packs both layouts so every DMA moves 1MB with
8KB-contiguous per-partition descriptors.
"""
import numpy as np
import ml_dtypes
from contextlib import ExitStack

import concourse.bacc as bacc
import concourse.tile as tile
import concourse.mybir as mybir
import concourse.masks as masks
from concourse.bass_utils import run_bass_kernel_spmd

F32 = mybir.dt.float32
BF16 = mybir.dt.bfloat16
ActFn = mybir.ActivationFunctionType

B, L, T = 4, 4096, 32
DH, DG, DP = 1024, 256, 256
WIN, STRIDE = 1024, 256
W = (L - WIN) // STRIDE + 1      # 13
SPAN = 2048                      # per-core l-span (disjoint)
NSLAB = 4                        # 512-l logits slabs
NDT = 8                          # d-tiles of 128
NCH = 8                          # 256-l chunks per core
NLT = 16                         # 128-l tiles per core
NGRP = 2                         # P output groups (4 chunks each, packed to 128 parts)

_CACHE = {}


def _build(with_mask: bool):
    nc = bacc.Bacc("TRN2", debug=False, target_bir_lowering=False)

    HT_d = nc.dram_tensor("HTl", [128, NSLAB * NDT * 512], BF16, kind="ExternalInput")
    Hn_d = nc.dram_tensor("Hnl", [128, 4 * 4 * DH], BF16, kind="ExternalInput")
    QKT_d = nc.dram_tensor("QKT", [128, NDT * T], BF16, kind="ExternalInput")
    if with_mask:
        mb_d = nc.dram_tensor("maskbias", [1, SPAN], BF16, kind="ExternalInput")
    P_d = nc.dram_tensor("P_out", [NGRP * 128, DH], BF16, kind="ExternalOutput")
    cs_d = nc.dram_tensor("csum_out", [T, NCH], F32, kind="ExternalOutput")

    with tile.TileContext(nc) as tc, ExitStack() as ctx:
        const = ctx.enter_context(tc.tile_pool(name="const", bufs=1))
        hpool = ctx.enter_context(tc.tile_pool(name="hpool", bufs=1))
        spool = ctx.enter_context(tc.tile_pool(name="spool", bufs=1))
        lg = ctx.enter_context(tc.tile_pool(name="lg", bufs=2, space="PSUM"))
        tp = ctx.enter_context(tc.tile_pool(name="tp", bufs=2, space="PSUM"))
        zp = ctx.enter_context(tc.tile_pool(name="zp", bufs=4, space="PSUM"))

        ident = const.tile([128, 128], F32, tag="ident")
        masks.make_identity(nc, ident[:])
        qkt = const.tile([128, NDT * T], BF16, tag="qkt")
        nc.sync.dma_start(qkt[:], QKT_d.ap())
        if with_mask:
            onesr = const.tile([1, T], BF16, tag="onesr")
            mbias = const.tile([1, SPAN], BF16, tag="mbias")
            nc.gpsimd.memset(onesr[:], 1.0)
            nc.sync.dma_start(mbias[:], mb_d.ap())

        ht = [hpool.tile([128, NDT * 512], BF16, tag=f"ht{s}", name=f"ht{s}")
              for s in range(NSLAB)]
        hn = [hpool.tile([128, 4 * DH], BF16, tag=f"hn{g}", name=f"hn{g}")
              for g in range(4)]
        for s in range(NSLAB):
            nc.sync.dma_start(ht[s][:], HT_d.ap()[:, s * 4096:(s + 1) * 4096])
        for g in range(4):
            nc.sync.dma_start(hn[g][:], Hn_d.ap()[:, g * 4096:(g + 1) * 4096])

        csum = spool.tile([T, NCH], F32, tag="csum")
        expL = [spool.tile([T, 512], F32, tag=f"expL{s}", name=f"expL{s}")
                for s in range(NSLAB)]
        expLT = [spool.tile([128, T], BF16, tag=f"eT{j}", name=f"eT{j}")
                 for j in range(NLT)]
        pstage = [spool.tile([128, DH], BF16, tag=f"pst{g}", name=f"pst{g}")
                  for g in range(NGRP)]

        zpt = {}
        for s in range(NSLAB):
            ps = lg.tile([T, 512], F32, tag="lg")
            for i in range(NDT):
                nc.tensor.matmul(ps[:], qkt[:, i * T:(i + 1) * T],
                                 ht[s][:, i * 512:(i + 1) * 512],
                                 start=(i == 0), stop=(i == NDT - 1 and not with_mask))
            if with_mask:
                nc.tensor.matmul(ps[:], onesr[:], mbias[:, s * 512:(s + 1) * 512],
                                 start=False, stop=True)
            for u in range(2):
                c = 2 * s + u
                nc.scalar.activation(expL[s][:, u * 256:(u + 1) * 256],
                                     ps[:, u * 256:(u + 1) * 256],
                                     ActFn.Exp, accum_out=csum[:, c:c + 1])
            for jj in range(4):
                j = 4 * s + jj
                tps = tp.tile([128, T], F32, tag="tp")
                nc.tensor.transpose(tps[:], expL[s][:, jj * 128:(jj + 1) * 128],
                                    ident[:T, :T])
                nc.vector.tensor_copy(expLT[j][:], tps[:])
            for u in range(2):
                c = 2 * s + u
                grp, q = c // 4, c % 4
                if q == 0:
                    zpt[(grp, 0)] = zp.tile([128, 512], F32, tag="zp",
                                            name=f"zp{grp}a")
                    zpt[(grp, 1)] = zp.tile([128, 512], F32, tag="zp",
                                            name=f"zp{grp}b")
                for lt in range(2):
                    j = 2 * c + lt
                    g2, j4 = j // 4, j % 4
                    for h in range(2):
                        nc.tensor.matmul(zpt[(grp, h)][q * 32:(q + 1) * 32, :],
                                         expLT[j][:],
                                         hn[g2][:, j4 * DH + h * 512:j4 * DH + (h + 1) * 512],
                                         start=(lt == 0), stop=(lt == 1),
                                         tile_position=(0, q * 32))
                if q == 3:
                    for h in range(2):
                        nc.vector.tensor_copy(pstage[grp][:, h * 512:(h + 1) * 512],
                                              zpt[(grp, h)][:])
                    nc.scalar.dma_start(P_d.ap()[grp * 128:(grp + 1) * 128, :],
                                        pstage[grp][:])
        nc.scalar.dma_start(cs_d.ap(), csum[:])

    nc.compile()
    return nc


def kernel(H, G, Wq_core, Wk_core, Wq_win, Wk_win, attn_mask):
    H = np.asarray(H, dtype=np.float32)
    G = np.asarray(G, dtype=np.float32)
    Wq_core = np.asarray(Wq_core, dtype=np.float32)
    Wk_core = np.asarray(Wk_core, dtype=np.float32)
    Wq_win = np.asarray(Wq_win, dtype=np.float32)
    Wk_win = np.asarray(Wk_win, dtype=np.float32)
    mask = np.asarray(attn_mask).astype(bool)

    with_mask = not bool(mask.all())
    key = ("k", with_mask)
    if key not in _CACHE:
        _CACHE[key] = _build(with_mask)
    nc = _CACHE[key]

    # host precompute of the tiny query-side projections (f64 for accuracy)
    G64 = G.astype(np.float64)
    QK = (G64 @ Wq_core.astype(np.float64)) @ Wk_core.astype(np.float64).T
    QK *= DP ** -0.5                                    # [B, T, DH]
    qw2 = (G64 @ Wq_win.astype(np.float64)) @ Wk_win.astype(np.float64).T
    qw2 *= DH ** -0.5                                   # [B, T, DH]

    Hb = H.astype(ml_dtypes.bfloat16)
    in_maps = []
    for c in range(8):
        b, half = c // 2, c % 2
        hs = Hb[b, half * SPAN:(half + 1) * SPAN]       # [2048, 1024] bf16
        Hn_l = np.ascontiguousarray(
            hs.reshape(4, 4, 128, DH).transpose(2, 0, 1, 3).reshape(128, 16384))
        HT_l = np.ascontiguousarray(
            hs.reshape(4, 512, 8, 128).transpose(3, 0, 2, 1).reshape(128, 16384))
        QKT_l = np.ascontiguousarray(
            QK[b].T.reshape(8, 128, T).transpose(1, 0, 2).reshape(128, 8 * T)
        ).astype(ml_dtypes.bfloat16)
        im = {"HTl": HT_l, "Hnl": Hn_l, "QKT": QKT_l}
        if with_mask:
            im["maskbias"] = np.where(
                mask[b, half * SPAN:(half + 1) * SPAN], 0.0, -1e9
            ).astype(ml_dtypes.bfloat16)[None, :]
        in_maps.append(im)

    import os
    prof_dir = os.environ.get("BGW_PROFILE_DIR")
    if prof_dir:
        res = run_bass_kernel_spmd(nc, in_maps, core_ids=list(range(8)),
                                   trace=True, tmpdir=prof_dir)
    else:
        res = run_bass_kernel_spmd(nc, in_maps, core_ids=list(range(8)))
    kernel._last_result = res

    # host combine: window sums of chunk partials, then tiny W=13 softmax
    Z = np.empty((B, T, DH), dtype=np.float32)
    for b in range(B):
        Pc, css = [], []
        for half in range(2):
            r = res.results[2 * b + half]
            Pc.append(np.asarray(r["P_out"]).astype(np.float32)
                      .reshape(NGRP * 4, T, DH))
            css.append(np.asarray(r["csum_out"]).astype(np.float32))
        P = np.concatenate(Pc, axis=0)                  # [16, T, DH]
        cs = np.concatenate(css, axis=1)                # [T, 16]
        S = P[0:13] + P[1:14] + P[2:15] + P[3:16]       # [13, T, DH]
        den = cs[:, 0:13] + cs[:, 1:14] + cs[:, 2:15] + cs[:, 3:16]   # [T, 13]
        Zw = S / den.T[:, :, None]                      # [13, T, DH]
        wlog = np.einsum('wtd,td->tw', Zw, qw2[b])
        wlog -= wlog.max(axis=1, keepdims=True)
        e = np.exp(wlog)
        wsm = e / e.sum(axis=1, keepdims=True)          # [T, 13]
        Z[b] = np.einsum('tw,wtd->td', wsm, Zw)
    return Z


# revision 8
# speedup vs baseline: 3.5780x; 1.0025x over previous
"""Trainium2 Bass kernel for nn_BucketedGoWatti (sparse windowed attention pooling).

Math (B=4, L=4096, T=32, DH=1024, DG=256, DP=256, WIN=1024, STRIDE=256, W=13):
  q  = G @ Wq_core;  logits[b,t,l] = (q @ Wk_core^T) . H[b,l]  (window-independent)
  alpha = softmax of logits restricted to window; Zw[b,t,w,:] = alpha @ Hw
  Since windows are 4 consecutive 256-chunks, Zw[w] = (P[w]+P[w+1]+P[w+2]+P[w+3])/den
  with P[c] = sum_{l in chunk c} exp(logit[t,l]) * H[l,:]  and den from per-chunk
  exp-sums. Device computes P[c] + csum[c] only; window composition, the tiny
  cross-window softmax (qw2 = (G@Wq_win)@Wk_win^T) and the final combine run on host.

Sharding: core c -> batch b=c//2, l-half c%2 (disjoint 2048 rows of H, zero halo).
Each core streams H once in each orientation (bf16): HT (d-major) for logits,
Hn (l-major) for P. Host pre-packs both layouts so every DMA moves 1MB with
8KB-contiguous per-partition descriptors.
"""
import numpy as np
import ml_dtypes
from contextlib import ExitStack

import concourse.bacc as bacc
import concourse.tile as tile
import concourse.mybir as mybir
import concourse.masks as masks
from concourse.bass_utils import run_bass_kernel_spmd

F32 = mybir.dt.float32
BF16 = mybir.dt.bfloat16
FP8 = mybir.dt.float8e4
ActFn = mybir.ActivationFunctionType

B, L, T = 4, 4096, 32
DH, DG, DP = 1024, 256, 256
WIN, STRIDE = 1024, 256
W = (L - WIN) // STRIDE + 1      # 13
SPAN = 2048                      # per-core l-span (disjoint)
NSLAB = 4                        # 512-l logits slabs
NDT = 8                          # d-tiles of 128
NCH = 8                          # 256-l chunks per core
NLT = 16                         # 128-l tiles per core
NGRP = 2                         # P output groups (4 chunks each, packed to 128 parts)

_CACHE = {}


def _build(with_mask: bool):
    nc = bacc.Bacc("TRN2", debug=False, target_bir_lowering=False)

    HT_d = nc.dram_tensor("HTl", [128, NSLAB * NDT * 512], FP8, kind="ExternalInput")
    Hn_d = nc.dram_tensor("Hnl", [128, 4 * 4 * DH], BF16, kind="ExternalInput")
    QKT_d = nc.dram_tensor("QKT", [128, NDT * T], FP8, kind="ExternalInput")
    if with_mask:
        mb_d = nc.dram_tensor("maskbias", [1, SPAN], FP8, kind="ExternalInput")
    P_d = nc.dram_tensor("P_out", [NGRP * 128, DH], BF16, kind="ExternalOutput")
    cs_d = nc.dram_tensor("csum_out", [T, NCH], F32, kind="ExternalOutput")

    with tile.TileContext(nc) as tc, ExitStack() as ctx:
        const = ctx.enter_context(tc.tile_pool(name="const", bufs=1))
        hpool = ctx.enter_context(tc.tile_pool(name="hpool", bufs=1))
        spool = ctx.enter_context(tc.tile_pool(name="spool", bufs=1))
        lg = ctx.enter_context(tc.tile_pool(name="lg", bufs=2, space="PSUM"))
        tp = ctx.enter_context(tc.tile_pool(name="tp", bufs=2, space="PSUM"))
        zp = ctx.enter_context(tc.tile_pool(name="zp", bufs=4, space="PSUM"))

        ident = const.tile([128, 128], F32, tag="ident")
        masks.make_identity(nc, ident[:])
        qkt = const.tile([128, NDT * T], FP8, tag="qkt")
        nc.scalar.dma_start(qkt[:], QKT_d.ap())
        if with_mask:
            onesr = const.tile([1, T], FP8, tag="onesr")
            mbias = const.tile([1, SPAN], FP8, tag="mbias")
            nc.gpsimd.memset(onesr[:], 1.0)
            nc.scalar.dma_start(mbias[:], mb_d.ap())

        ht = [hpool.tile([128, NDT * 512], FP8, tag=f"ht{s}", name=f"ht{s}")
              for s in range(NSLAB)]
        hn = [hpool.tile([128, 4 * DH], BF16, tag=f"hn{g}", name=f"hn{g}")
              for g in range(4)]
        for s in range(NSLAB):
            nc.sync.dma_start(ht[s][:], HT_d.ap()[:, s * 4096:(s + 1) * 4096])
        for g in range(4):
            nc.sync.dma_start(hn[g][:], Hn_d.ap()[:, g * 4096:(g + 1) * 4096])

        csum = spool.tile([T, NCH], F32, tag="csum")
        expL = [spool.tile([T, 512], F32, tag=f"expL{s}", name=f"expL{s}")
                for s in range(NSLAB)]
        expLT = [spool.tile([128, T], BF16, tag=f"eT{j}", name=f"eT{j}")
                 for j in range(NLT)]
        pstage = [spool.tile([128, DH], BF16, tag=f"pst{g}", name=f"pst{g}")
                  for g in range(NGRP)]

        # pass 1: logits -> exp -> transpose, per 512-l slab (gated by ht DMAs)
        for s in range(NSLAB):
            ps = lg.tile([T, 512], F32, tag="lg")
            for i in range(NDT):
                nc.tensor.matmul(ps[:], qkt[:, i * T:(i + 1) * T],
                                 ht[s][:, i * 512:(i + 1) * 512],
                                 start=(i == 0), stop=(i == NDT - 1 and not with_mask))
            if with_mask:
                nc.tensor.matmul(ps[:], onesr[:], mbias[:, s * 512:(s + 1) * 512],
                                 start=False, stop=True)
            for u in range(2):
                c = 2 * s + u
                nc.scalar.activation(expL[s][:, u * 256:(u + 1) * 256],
                                     ps[:, u * 256:(u + 1) * 256],
                                     ActFn.Exp, accum_out=csum[:, c:c + 1])
            for jj in range(4):
                j = 4 * s + jj
                tps = tp.tile([128, T], F32, tag="tp")
                nc.tensor.transpose(tps[:], expL[s][:, jj * 128:(jj + 1) * 128],
                                    ident[:T, :T])
                nc.vector.tensor_copy(expLT[j][:], tps[:])

        # csum is final once all exps ran; ship it early, off the tail
        nc.scalar.dma_start(cs_d.ap(), csum[:])

        # pass 2: P chunks (gated by hn DMAs), packed 4 chunks -> 128 partitions
        zpt = {}
        for c in range(NCH):
            grp, q = c // 4, c % 4
            if q == 0:
                zpt[(grp, 0)] = zp.tile([128, 512], F32, tag="zp",
                                        name=f"zp{grp}a")
                zpt[(grp, 1)] = zp.tile([128, 512], F32, tag="zp",
                                        name=f"zp{grp}b")
            for lt in range(2):
                j = 2 * c + lt
                g2, j4 = j // 4, j % 4
                for h in range(2):
                    nc.tensor.matmul(zpt[(grp, h)][q * 32:(q + 1) * 32, :],
                                     expLT[j][:],
                                     hn[g2][:, j4 * DH + h * 512:j4 * DH + (h + 1) * 512],
                                     start=(lt == 0), stop=(lt == 1),
                                     tile_position=(0, q * 32))
            if q == 3:
                nc.vector.tensor_copy(pstage[grp][:, 0:512], zpt[(grp, 0)][:])
                nc.scalar.activation(pstage[grp][:, 512:1024], zpt[(grp, 1)][:],
                                     ActFn.Copy)
                nc.scalar.dma_start(P_d.ap()[grp * 128:(grp + 1) * 128, :],
                                    pstage[grp][:])

    nc.compile()
    return nc


def kernel(H, G, Wq_core, Wk_core, Wq_win, Wk_win, attn_mask):
    H = np.asarray(H, dtype=np.float32)
    G = np.asarray(G, dtype=np.float32)
    Wq_core = np.asarray(Wq_core, dtype=np.float32)
    Wk_core = np.asarray(Wk_core, dtype=np.float32)
    Wq_win = np.asarray(Wq_win, dtype=np.float32)
    Wk_win = np.asarray(Wk_win, dtype=np.float32)
    mask = np.asarray(attn_mask).astype(bool)

    with_mask = not bool(mask.all())
    key = ("k", with_mask)
    if key not in _CACHE:
        _CACHE[key] = _build(with_mask)
    nc = _CACHE[key]

    # host precompute of the tiny query-side projections (f64 for accuracy)
    G64 = G.astype(np.float64)
    QK = (G64 @ Wq_core.astype(np.float64)) @ Wk_core.astype(np.float64).T
    QK *= DP ** -0.5                                    # [B, T, DH]
    qw2 = (G64 @ Wq_win.astype(np.float64)) @ Wk_win.astype(np.float64).T
    qw2 *= DH ** -0.5                                   # [B, T, DH]

    Hb = H.astype(ml_dtypes.bfloat16)
    H8 = H.astype(ml_dtypes.float8_e4m3fn)
    in_maps = []
    for c in range(8):
        b, half = c // 2, c % 2
        hs = Hb[b, half * SPAN:(half + 1) * SPAN]       # [2048, 1024] bf16
        h8 = H8[b, half * SPAN:(half + 1) * SPAN]       # [2048, 1024] fp8
        Hn_l = np.ascontiguousarray(
            hs.reshape(4, 4, 128, DH).transpose(2, 0, 1, 3).reshape(128, 16384))
        HT_l = np.ascontiguousarray(
            h8.reshape(4, 512, 8, 128).transpose(3, 0, 2, 1).reshape(128, 16384))
        QKT_l = np.ascontiguousarray(
            QK[b].T.reshape(8, 128, T).transpose(1, 0, 2).reshape(128, 8 * T)
        ).astype(ml_dtypes.float8_e4m3fn)
        im = {"HTl": HT_l, "Hnl": Hn_l, "QKT": QKT_l}
        if with_mask:
            im["maskbias"] = np.where(
                mask[b, half * SPAN:(half + 1) * SPAN], 0.0, -448.0
            ).astype(ml_dtypes.float8_e4m3fn)[None, :]
        in_maps.append(im)

    import os
    prof_dir = os.environ.get("BGW_PROFILE_DIR")
    if prof_dir:
        res = run_bass_kernel_spmd(nc, in_maps, core_ids=list(range(8)),
                                   trace=True, tmpdir=prof_dir)
    else:
        res = run_bass_kernel_spmd(nc, in_maps, core_ids=list(range(8)))
    kernel._last_result = res

    # host combine: window sums of chunk partials, then tiny W=13 softmax
    Z = np.empty((B, T, DH), dtype=np.float32)
    for b in range(B):
        Pc, css = [], []
        for half in range(2):
            r = res.results[2 * b + half]
            Pc.append(np.asarray(r["P_out"]).astype(np.float32)
                      .reshape(NGRP * 4, T, DH))
            css.append(np.asarray(r["csum_out"]).astype(np.float32))
        P = np.concatenate(Pc, axis=0)                  # [16, T, DH]
        cs = np.concatenate(css, axis=1)                # [T, 16]
        S = P[0:13] + P[1:14] + P[2:15] + P[3:16]       # [13, T, DH]
        den = cs[:, 0:13] + cs[:, 1:14] + cs[:, 2:15] + cs[:, 3:16]   # [T, 13]
        Zw = S / den.T[:, :, None]                      # [13, T, DH]
        wlog = np.einsum('wtd,td->tw', Zw, qw2[b])
        wlog -= wlog.max(axis=1, keepdims=True)
        e = np.exp(wlog)
        wsm = e / e.sum(axis=1, keepdims=True)          # [T, 13]
        Z[b] = np.einsum('tw,wtd->td', wsm, Zw)
    return Z
